# revision 7
# baseline (speedup 1.0000x reference)
"""Mistral sparse-MoE (B=4,S=2048,H=1024,F=4096,E=8,top-2) on 8 trn2 cores.

Expert-parallel sharding: core e holds expert e's gate/up/down weights.
The host computes the (tiny) router + top-2 dispatch and uses it to shard:
each core receives exactly the tokens routed to its expert (gathered,
transposed, zero-padded to a common capacity C), the expert weights in
K-major partition-blocked bf16 layout, and the per-token combine weights.
The device kernel computes the full expert FFN
  y = (silu(x@gW^T) * (x@uW^T)) @ dW^T * w
for its tokens; the host scatter-adds the 8 partial outputs back into the
[T, H] result (pure unshard of the expert-parallel partial sums).

DRAM layouts are partition-major ([128, k, free]) so every DMA is a single
contiguous-run-per-partition access pattern.
"""

import numpy as np
import ml_dtypes
from contextlib import ExitStack

B, S, H, F, E, TOPK = 4, 2048, 1024, 4096, 8, 2
T = B * S
P = 128
NCH = 512          # token chunk (columns per psum tile)
FQ = 1024          # f-columns loaded per gate/up weight DMA
KH = H // P        # 8  contraction chunks for gate/up
KF = F // P        # 32 contraction chunks for down
HM = H // P        # 8  output row tiles

_BF16 = ml_dtypes.bfloat16


def _build_program(C, repeat=1):
    import concourse.tile as tile
    from concourse import bacc, mybir

    bf16 = mybir.dt.bfloat16
    f32 = mybir.dt.float32

    nc = bacc.Bacc("TRN2", target_bir_lowering=False, debug=False, num_devices=E)

    xT = nc.dram_tensor("xT", [P, KH, C], bf16, kind="ExternalInput").ap()
    gw = nc.dram_tensor("gw", [P, KH, F], bf16, kind="ExternalInput").ap()
    uw = nc.dram_tensor("uw", [P, KH, F], bf16, kind="ExternalInput").ap()
    dw = nc.dram_tensor("dw", [P, KF, H], bf16, kind="ExternalInput").ap()
    wr = nc.dram_tensor("wr", [P, C], f32, kind="ExternalInput").ap()
    yT = nc.dram_tensor("yT", [P, HM, C], bf16, kind="ExternalOutput").ap()

    # balanced chunk sizes (all ~C/ceil(C/NCH)): a small tail chunk would
    # have too little compute to hide its 16.8MB gate/up weight reload
    n_chunks = (C + NCH - 1) // NCH
    base, extra = divmod(C, n_chunks)
    chunks = []
    n0 = 0
    for i in range(n_chunks):
        nn = base + (1 if i < extra else 0)
        chunks.append((n0, nn))
        n0 += nn

    with tile.TileContext(nc) as tc, ExitStack() as ctx:
        dwp = ctx.enter_context(tc.tile_pool(name="dwp", bufs=1))
        wp = ctx.enter_context(tc.tile_pool(name="wp", bufs=1))
        xp = ctx.enter_context(tc.tile_pool(name="xp", bufs=2))
        gwp = ctx.enter_context(tc.tile_pool(name="gwp", bufs=2))
        uwp = ctx.enter_context(tc.tile_pool(name="uwp", bufs=2))
        hp = ctx.enter_context(tc.tile_pool(name="hp", bufs=1))
        sgp = ctx.enter_context(tc.tile_pool(name="sgp", bufs=4))
        yp = ctx.enter_context(tc.tile_pool(name="yp", bufs=2))
        pg = ctx.enter_context(tc.tile_pool(name="pg", bufs=3, space="PSUM"))
        pu = ctx.enter_context(tc.tile_pool(name="pu", bufs=3, space="PSUM"))
        py = ctx.enter_context(tc.tile_pool(name="py", bufs=2, space="PSUM"))

        for rep in range(repeat):
            # down-proj weights resident: one [128, 32, 1024] tile, single DMA
            dwt = dwp.tile([P, KF, H], bf16)
            nc.sync.dma_start(out=dwt[:], in_=dw[:, :, :])
            wt = wp.tile([P, C], f32)
            nc.sync.dma_start(out=wt[:], in_=wr[:, :])

            for (n0, nn) in chunks:
                xt = xp.tile([P, KH, nn], bf16)
                nc.sync.dma_start(out=xt[:], in_=xT[:, :, n0:n0 + nn])

                hts = []
                for q in range(F // FQ):
                    f0 = q * FQ
                    gt = gwp.tile([P, KH, FQ], bf16)
                    nc.sync.dma_start(out=gt[:], in_=gw[:, :, f0:f0 + FQ])
                    ut = uwp.tile([P, KH, FQ], bf16)
                    nc.sync.dma_start(out=ut[:], in_=uw[:, :, f0:f0 + FQ])
                    for fm in range(FQ // P):
                        j = q * (FQ // P) + fm
                        psg = pg.tile([P, nn], f32)
                        psu = pu.tile([P, nn], f32)
                        for k in range(KH):
                            nc.tensor.matmul(
                                psg[:], gt[:, k, fm * P:(fm + 1) * P], xt[:, k, :],
                                start=(k == 0), stop=(k == KH - 1))
                        for k in range(KH):
                            nc.tensor.matmul(
                                psu[:], ut[:, k, fm * P:(fm + 1) * P], xt[:, k, :],
                                start=(k == 0), stop=(k == KH - 1))
                        sg = sgp.tile([P, nn], bf16)
                        nc.scalar.activation(
                            sg[:], psg[:], mybir.ActivationFunctionType.Silu)
                        ht = hp.tile([P, nn], bf16, tag=f"h{j}")
                        nc.vector.tensor_mul(ht[:], sg[:], psu[:])
                        hts.append(ht)

                yt = yp.tile([P, HM, nn], bf16)
                for hm in range(HM):
                    psy = py.tile([P, nn], f32)
                    for k in range(KF):
                        nc.tensor.matmul(
                            psy[:], dwt[:, k, hm * P:(hm + 1) * P], hts[k][:],
                            start=(k == 0), stop=(k == KF - 1))
                    nc.vector.tensor_mul(yt[:, hm, :], psy[:], wt[:, n0:n0 + nn])
                nc.sync.dma_start(out=yT[:, :, n0:n0 + nn], in_=yt[:])

    nc.finalize()
    return nc


def _route(x, router_w):
    # top-2 routing in f64 (exactly ties-stable vs the fp32 reference for
    # any non-degenerate logits)
    logits = x.astype(np.float64) @ router_w.T.astype(np.float64)
    rows = np.arange(T)
    i1 = np.argmax(logits, axis=1)
    v1 = logits[rows, i1]
    masked = logits.copy()
    masked[rows, i1] = -np.inf
    i2 = np.argmax(masked, axis=1)
    v2 = masked[rows, i2]
    e2 = np.exp(v2 - v1)
    w1 = 1.0 / (1.0 + e2)
    w2 = e2 / (1.0 + e2)
    return i1, i2, w1.astype(np.float32), w2.astype(np.float32)


def _pmajor(a, kdim):
    """[K*128, N] -> [128, K, N] partition-major contiguous."""
    k, n = a.shape
    return np.ascontiguousarray(
        a.reshape(kdim, P, n).transpose(1, 0, 2))


def kernel(hidden_states, router_w, gate_w, up_w, down_w):
    from concourse.bass_utils import run_bass_kernel_spmd

    x = np.asarray(hidden_states, dtype=np.float32).reshape(T, H)
    router_w = np.asarray(router_w, dtype=np.float32)

    i1, i2, w1, w2 = _route(x, router_w)

    # NOTE: capacity-limited routing (cap 2048, drop lowest-weight overflow)
    # was evaluated and REJECTED: this router's top-2 softmax weights are
    # broadly distributed (dropped w up to 0.24), giving rel_l2 3.4e-2 >
    # the 2e-2 budget for only a 1.4% capacity saving.
    idxs, wts = [], []
    for e in range(E):
        m1 = i1 == e
        m2 = i2 == e
        idx = np.nonzero(m1 | m2)[0]
        w = np.where(m1[idx], w1[idx], w2[idx])
        idxs.append(idx)
        wts.append(w)

    max_ne = max(len(i) for i in idxs)
    C = max(NCH, max_ne)

    x_bf = x.astype(_BF16)
    in_maps = []
    for e in range(E):
        idx, w = idxs[e], wts[e]
        n_e = len(idx)
        xTe = np.zeros((H, C), dtype=_BF16)
        xTe[:, :n_e] = x_bf[idx].T
        wre = np.zeros((P, C), dtype=np.float32)
        wre[:, :n_e] = w[None, :]
        in_maps.append({
            "xT": _pmajor(xTe, KH),
            "gw": _pmajor(np.asarray(gate_w)[e].T.astype(_BF16), KH),
            "uw": _pmajor(np.asarray(up_w)[e].T.astype(_BF16), KH),
            "dw": _pmajor(np.asarray(down_w)[e].T.astype(_BF16), KF),
            "wr": wre,
        })

    nc = _build_program(C)
    results = run_bass_kernel_spmd(nc, in_maps, list(range(E))).results

    out = np.zeros((T, H), dtype=np.float32)
    for e in range(E):
        idx = idxs[e]
        # yT dram is [128, HM, C] partition-major -> [H, C]
        yTe = results[e]["yT"].transpose(1, 0, 2).reshape(H, C)
        out[idx] += yTe[:, :len(idx)].astype(np.float32).T
    return out.reshape(B, S, H)

